# revision 2
# baseline (speedup 1.0000x reference)
"""CRF mean-field (nn_CRF) Trainium2 kernel, SPMD over 8 NeuronCores.

Math: 5 iterations of
    p   = softmax(q, axis=classes)
    out_f = p @ K_f           for two Gaussian kernels K_f (spatial, bilateral)
    q   = unaries - compat @ (sw @ out_sp + bw @ out_bl)

Sharding: points N=8192 split column-wise across 8 cores (1024 each). Each core
rebuilds its [8192, 1024] kernel slab on the fly every iteration (cheaper than
streaming a precomputed slab from HBM): the PE computes the full Gaussian
exponent -0.5*||f_i - f_j||^2 directly via an augmented feature matmul
    lhsT rows = [f ; -0.5*|f|^2 ; 1],  rhs rows = [f ; 1 ; -0.5*|f|^2]
ScalarE exponentiates straight out of PSUM, and the tile is immediately
consumed as the rhs of the accumulating p @ K matmul. Per iteration the cores
exchange only their local q.T shard (40KB) via AllGather.
"""

import numpy as np

C = 10          # classes
N = 8192        # points
S = 3           # spatial dims
R = 8           # cores
NL = N // R     # local points per core
KCH = N // 128  # 64 i-chunks
JCH = NL // 128  # 8 local j-chunks
NITER = 5
THETA_GAMMA = 8.0

_CACHE = {}


def _build_program():
    import concourse.mybir as mybir
    import concourse.tile as tile
    from concourse import bacc
    from concourse.bass import ts, ds

    f32 = mybir.dt.float32
    nc = bacc.Bacc("TRN2", target_bir_lowering=False, debug=False, num_devices=R)

    # ---- I/O ----
    fstack_full = nc.dram_tensor("fstack_full", [40, N], f32, kind="ExternalInput")
    fstack_loc = nc.dram_tensor("fstack_loc", [40, NL], f32, kind="ExternalInput")
    qT_init = nc.dram_tensor("qT_init", [N, C], f32, kind="ExternalInput")
    unT_loc = nc.dram_tensor("unT_loc", [NL, C], f32, kind="ExternalInput")
    amT_sp = nc.dram_tensor("amT_sp", [C, C], f32, kind="ExternalInput")
    amT_bl = nc.dram_tensor("amT_bl", [C, C], f32, kind="ExternalInput")
    qT_out = nc.dram_tensor("qT_out", [NL, C], f32, kind="ExternalOutput")

    EXP = mybir.ActivationFunctionType.Exp

    with tile.TileContext(nc) as tc:
        with (
            tc.tile_pool(name="const", bufs=1) as const,
            tc.tile_pool(name="state", bufs=1) as state,
            tc.tile_pool(name="epool", bufs=4) as epool,
            tc.tile_pool(name="opool", bufs=2) as opool,
            tc.tile_pool(name="qpool", bufs=2) as qpool,
            tc.tile_pool(name="psG", bufs=2, space="PSUM") as psG,
            tc.tile_pool(name="psOs", bufs=1, space="PSUM") as psOs,
            tc.tile_pool(name="psOb", bufs=1, space="PSUM") as psOb,
            tc.tile_pool(name="dram", bufs=2, space="DRAM") as dram,
        ):
            # ---- load constants ----
            ff_sb = const.tile([40, N], f32, name="ff_sb")
            fl_sb = const.tile([40, NL], f32, name="fl_sb")
            amT_sp_sb = const.tile([C, C], f32, name="amT_sp_sb")
            amT_bl_sb = const.tile([C, C], f32, name="amT_bl_sb")
            un_sb = const.tile([128, JCH, C], f32, name="un_sb")
            nc.sync.dma_start(ff_sb[:], fstack_full[:])
            nc.sync.dma_start(fl_sb[:], fstack_loc[:])
            nc.sync.dma_start(amT_sp_sb[:], amT_sp[:])
            nc.sync.dma_start(amT_bl_sb[:], amT_bl[:])
            nc.sync.dma_start(
                un_sb[:], unT_loc[:].rearrange("(j p) c -> p j c", p=128)
            )

            q_sb = state.tile([128, KCH, C], f32, name="q_sb")
            p_sb = state.tile([128, KCH, C], f32, name="p_sb")
            e_sb = state.tile([128, KCH, C], f32, name="e_sb")
            mx_sb = state.tile([128, KCH], f32, name="mx_sb")
            sm_sb = state.tile([128, KCH], f32, name="sm_sb")
            rs_sb = state.tile([128, KCH], f32, name="rs_sb")
            nc.sync.dma_start(q_sb[:], qT_init[:].rearrange("(k p) c -> p k c", p=128))

            for t in range(NITER):
                # ---- softmax over classes (innermost free dim) ----
                nc.vector.reduce_max(mx_sb[:], q_sb[:], axis=mybir.AxisListType.X)
                mx_b = mx_sb[:].unsqueeze(2).broadcast_to((128, KCH, C))
                nc.vector.tensor_sub(e_sb[:], q_sb[:], mx_b)
                nc.scalar.activation(e_sb[:], e_sb[:], EXP, bias=0.0, scale=1.0)
                nc.vector.reduce_sum(sm_sb[:], e_sb[:], axis=mybir.AxisListType.X)
                nc.vector.reciprocal(rs_sb[:], sm_sb[:])
                rs_b = rs_sb[:].unsqueeze(2).broadcast_to((128, KCH, C))
                nc.vector.tensor_mul(p_sb[:], e_sb[:], rs_b)

                # ---- out[10, NL] = p.T-slab matmuls with fused kernel build ----
                po_s = psOs.tile([C, NL], f32, name="po_s")
                po_b = psOb.tile([C, NL], f32, name="po_b")
                for k in range(KCH):
                    for h in range(2):
                        jsl = ds(h * 512, 512)
                        gt = psG.tile([128, 1024], f32, name="gt", tag="gt")
                        nc.tensor.matmul(
                            gt[:, 0:512],
                            ff_sb[0:5, ts(k, 128)],
                            fl_sb[0:5, jsl],
                            start=True, stop=True,
                        )
                        nc.tensor.matmul(
                            gt[:, 512:1024],
                            ff_sb[32:40, ts(k, 128)],
                            fl_sb[32:40, jsl],
                            start=True, stop=True,
                        )
                        et = epool.tile([128, 1024], f32, name="et")
                        nc.scalar.activation(et[:], gt[:], EXP, bias=0.0, scale=1.0)
                        nc.tensor.matmul(
                            po_s[:, jsl],
                            p_sb[:, k, :],
                            et[:, 0:512],
                            start=(k == 0), stop=(k == KCH - 1),
                        )
                        nc.tensor.matmul(
                            po_b[:, jsl],
                            p_sb[:, k, :],
                            et[:, 512:1024],
                            start=(k == 0), stop=(k == KCH - 1),
                        )

                # ---- q_loc.T = unT_loc + out_sp.T @ amT_sp + out_bl.T @ amT_bl ----
                ot_s = opool.tile([C, NL], f32, name="ot_s")
                ot_b = opool.tile([C, NL], f32, name="ot_b")
                nc.vector.tensor_copy(ot_s[:], po_s[:])
                nc.vector.tensor_copy(ot_b[:], po_b[:])
                qa = psG.tile([128, JCH, C], f32, name="qa", tag="gt")
                for j in range(JCH):
                    nc.tensor.matmul(
                        qa[:, j, :], ot_s[:, ts(j, 128)], amT_sp_sb[:],
                        start=True, stop=False,
                    )
                    nc.tensor.matmul(
                        qa[:, j, :], ot_b[:, ts(j, 128)], amT_bl_sb[:],
                        start=False, stop=True,
                    )
                ql = qpool.tile([128, JCH, C], f32, name="ql")
                nc.vector.tensor_add(ql[:], qa[:], un_sb[:])

                if t < NITER - 1:
                    bi = dram.tile([NL, C], f32, name="bi")
                    bo = dram.tile([N, C], f32, addr_space="Shared", name="bo")
                    nc.sync.dma_start(
                        bi[:].rearrange("(j p) c -> p j c", p=128), ql[:]
                    )
                    nc.gpsimd.collective_compute(
                        "AllGather",
                        mybir.AluOpType.bypass,
                        replica_groups=[list(range(R))],
                        ins=[bi[:].opt()],
                        outs=[bo[:].opt()],
                    )
                    nc.sync.dma_start(
                        q_sb[:], bo[:].rearrange("(k p) c -> p k c", p=128)
                    )
                else:
                    nc.sync.dma_start(
                        qT_out[:].rearrange("(j p) c -> p j c", p=128), ql[:]
                    )

    nc.compile()
    return nc


def _get_program():
    if "nc" not in _CACHE:
        _CACHE["nc"] = _build_program()
    return _CACHE["nc"]


def _host_prep(unaries, feat, sw, bw, compat):
    f_sp = feat[:S] / THETA_GAMMA
    f_bl = feat
    f2_sp = np.sum(f_sp * f_sp, axis=0)
    f2_bl = np.sum(f_bl * f_bl, axis=0)

    fstack_full = np.zeros((40, N), dtype=np.float32)
    fstack_full[0:S] = f_sp
    fstack_full[S] = -0.5 * f2_sp
    fstack_full[S + 1] = 1.0
    fstack_full[32:38] = f_bl
    fstack_full[38] = -0.5 * f2_bl
    fstack_full[39] = 1.0

    fstack_loc_full = np.zeros((40, N), dtype=np.float32)
    fstack_loc_full[0:S] = f_sp
    fstack_loc_full[S] = 1.0
    fstack_loc_full[S + 1] = -0.5 * f2_sp
    fstack_loc_full[32:38] = f_bl
    fstack_loc_full[38] = 1.0
    fstack_loc_full[39] = -0.5 * f2_bl

    amT_sp = np.ascontiguousarray((-(compat @ sw)).T).astype(np.float32)
    amT_bl = np.ascontiguousarray((-(compat @ bw)).T).astype(np.float32)
    qT_init = np.ascontiguousarray(unaries.T).astype(np.float32)
    return fstack_full, fstack_loc_full, amT_sp, amT_bl, qT_init


def kernel(unaries, feat, spatial_weights, bilateral_weights, compatibility_matrix):
    from concourse.bass_utils import run_bass_kernel_spmd

    unaries = np.asarray(unaries, dtype=np.float32)
    feat = np.asarray(feat, dtype=np.float32)
    sw = np.asarray(spatial_weights, dtype=np.float32)
    bw = np.asarray(bilateral_weights, dtype=np.float32)
    compat = np.asarray(compatibility_matrix, dtype=np.float32)

    fstack_full, fstack_loc_full, amT_sp, amT_bl, qT_init = _host_prep(
        unaries, feat, sw, bw, compat
    )

    nc = _get_program()
    in_maps = []
    for r in range(R):
        jsl = slice(r * NL, (r + 1) * NL)
        in_maps.append({
            "fstack_full": fstack_full,
            "fstack_loc": np.ascontiguousarray(fstack_loc_full[:, jsl]),
            "qT_init": qT_init,
            "unT_loc": np.ascontiguousarray(qT_init[jsl]),
            "amT_sp": amT_sp,
            "amT_bl": amT_bl,
        })

    res = run_bass_kernel_spmd(nc, in_maps, core_ids=list(range(R)))

    q = np.empty((C, N), dtype=np.float32)
    for r in range(R):
        q[:, r * NL:(r + 1) * NL] = res.results[r]["qT_out"].T
    return q


# revision 3
# speedup vs baseline: 1.8711x; 1.8711x over previous
"""CRF mean-field (nn_CRF) Trainium2 kernel, SPMD over 8 NeuronCores.

Math: 5 iterations of
    p   = softmax(q, axis=classes)
    out_f = p @ K_f           for two Gaussian kernels K_f (spatial, bilateral)
    q   = unaries - compat @ (sw @ out_sp + bw @ out_bl)

Sharding: points N=8192 split column-wise across 8 cores (1024 each). Each core
rebuilds its [8192, 1024] kernel slab on the fly every iteration (cheaper than
streaming a precomputed slab from HBM): the PE computes the partial Gaussian
exponent  G'' = f_i . f_j - 0.5|f_i|^2  via an augmented feature matmul
(lhsT rows = [f ; -0.5|f|^2], rhs rows = [f ; 1]), ScalarE exponentiates
straight out of PSUM, and the tile is immediately consumed as the rhs of the
accumulating p @ K matmul. The remaining exp(-0.5|f_j|^2) factor is an exact
fp32 per-column post-scale. Heavy matmuls run in float32r (1 cyc/row vs 4 for
fp32); the j-side factor stays fp32 so fp32r rounding only enters through
terms that average out over the 8192-point contraction. Per iteration the
cores exchange only their local q.T shard (40KB) via AllGather.
"""

import numpy as np

C = 10          # classes
N = 8192        # points
S = 3           # spatial dims
R = 8           # cores
NL = N // R     # local points per core
KCH = N // 128  # 64 i-chunks
JCH = NL // 128  # 8 local j-chunks
NITER = 5
THETA_GAMMA = 8.0

_CACHE = {}


def _build_program():
    import concourse.mybir as mybir
    import concourse.tile as tile
    from concourse import bacc
    from concourse.bass import ts, ds

    f32 = mybir.dt.float32
    f32r = mybir.dt.float32r
    nc = bacc.Bacc("TRN2", target_bir_lowering=False, debug=False, num_devices=R)

    # ---- I/O ----
    fstack_full = nc.dram_tensor("fstack_full", [39, N], f32, kind="ExternalInput")
    fstack_loc = nc.dram_tensor("fstack_loc", [39, NL], f32, kind="ExternalInput")
    qT_init = nc.dram_tensor("qT_init", [N, C], f32, kind="ExternalInput")
    unT_loc = nc.dram_tensor("unT_loc", [NL, C], f32, kind="ExternalInput")
    amT_sp = nc.dram_tensor("amT_sp", [C, C], f32, kind="ExternalInput")
    amT_bl = nc.dram_tensor("amT_bl", [C, C], f32, kind="ExternalInput")
    arep_sp = nc.dram_tensor("arep_sp", [C, NL], f32, kind="ExternalInput")
    arep_bl = nc.dram_tensor("arep_bl", [C, NL], f32, kind="ExternalInput")
    qT_out = nc.dram_tensor("qT_out", [NL, C], f32, kind="ExternalOutput")

    EXP = mybir.ActivationFunctionType.Exp

    with tile.TileContext(nc) as tc:
        with (
            tc.tile_pool(name="const", bufs=1) as const,
            tc.tile_pool(name="state", bufs=1) as state,
            tc.tile_pool(name="epool", bufs=4) as epool,
            tc.tile_pool(name="opool", bufs=2) as opool,
            tc.tile_pool(name="qpool", bufs=2) as qpool,
            tc.tile_pool(name="psG", bufs=2, space="PSUM") as psG,
            tc.tile_pool(name="psOs", bufs=1, space="PSUM") as psOs,
            tc.tile_pool(name="psOb", bufs=1, space="PSUM") as psOb,
            tc.tile_pool(name="dram", bufs=2, space="DRAM") as dram,
        ):
            # ---- load constants; round feature stacks to f32r on-chip ----
            ff_st = const.tile([39, N], f32, name="ff_st")
            fl_st = const.tile([39, NL], f32, name="fl_st")
            ff_sb = const.tile([39, N], f32r, name="ff_sb")
            fl_sb = const.tile([39, NL], f32r, name="fl_sb")
            amT_sp_sb = const.tile([C, C], f32, name="amT_sp_sb")
            amT_bl_sb = const.tile([C, C], f32, name="amT_bl_sb")
            arep_sp_sb = const.tile([C, NL], f32, name="arep_sp_sb")
            arep_bl_sb = const.tile([C, NL], f32, name="arep_bl_sb")
            un_sb = const.tile([128, JCH, C], f32, name="un_sb")
            nc.sync.dma_start(ff_st[:], fstack_full[:])
            nc.sync.dma_start(fl_st[:], fstack_loc[:])
            nc.vector.tensor_copy(ff_sb[:], ff_st[:])
            nc.vector.tensor_copy(fl_sb[:], fl_st[:])
            nc.sync.dma_start(amT_sp_sb[:], amT_sp[:])
            nc.sync.dma_start(amT_bl_sb[:], amT_bl[:])
            nc.sync.dma_start(arep_sp_sb[:], arep_sp[:])
            nc.sync.dma_start(arep_bl_sb[:], arep_bl[:])
            nc.sync.dma_start(
                un_sb[:], unT_loc[:].rearrange("(j p) c -> p j c", p=128)
            )

            q_sb = state.tile([128, KCH, C], f32, name="q_sb")
            p_sb = state.tile([128, KCH, C], f32r, name="p_sb")
            e_sb = state.tile([128, KCH, C], f32, name="e_sb")
            mx_sb = state.tile([128, KCH], f32, name="mx_sb")
            sm_sb = state.tile([128, KCH], f32, name="sm_sb")
            rs_sb = state.tile([128, KCH], f32, name="rs_sb")
            nc.sync.dma_start(q_sb[:], qT_init[:].rearrange("(k p) c -> p k c", p=128))

            for t in range(NITER):
                # ---- softmax over classes (innermost free dim) ----
                nc.vector.reduce_max(mx_sb[:], q_sb[:], axis=mybir.AxisListType.X)
                mx_b = mx_sb[:].unsqueeze(2).broadcast_to((128, KCH, C))
                nc.vector.tensor_sub(e_sb[:], q_sb[:], mx_b)
                nc.scalar.activation(e_sb[:], e_sb[:], EXP, bias=0.0, scale=1.0)
                nc.vector.reduce_sum(sm_sb[:], e_sb[:], axis=mybir.AxisListType.X)
                nc.vector.reciprocal(rs_sb[:], sm_sb[:])
                rs_b = rs_sb[:].unsqueeze(2).broadcast_to((128, KCH, C))
                nc.vector.tensor_mul(p_sb[:], e_sb[:], rs_b)

                # ---- out[10, NL] = p.T-slab matmuls with fused kernel build ----
                po_s = psOs.tile([C, NL], f32, name="po_s")
                po_b = psOb.tile([C, NL], f32, name="po_b")
                for k in range(KCH):
                    for h in range(2):
                        jsl = ds(h * 512, 512)
                        gt = psG.tile([128, 1024], f32, name="gt", tag="gt")
                        nc.tensor.matmul(
                            gt[:, 0:512],
                            ff_sb[0:4, ts(k, 128)],
                            fl_sb[0:4, jsl],
                            start=True, stop=True,
                        )
                        nc.tensor.matmul(
                            gt[:, 512:1024],
                            ff_sb[32:39, ts(k, 128)],
                            fl_sb[32:39, jsl],
                            start=True, stop=True,
                        )
                        et = epool.tile([128, 1024], f32r, name="et")
                        nc.scalar.activation(et[:], gt[:], EXP, bias=0.0, scale=1.0)
                        nc.tensor.matmul(
                            po_s[:, jsl],
                            p_sb[:, k, :],
                            et[:, 0:512],
                            start=(k == 0), stop=(k == KCH - 1),
                        )
                        nc.tensor.matmul(
                            po_b[:, jsl],
                            p_sb[:, k, :],
                            et[:, 512:1024],
                            start=(k == 0), stop=(k == KCH - 1),
                        )

                # ---- q_loc.T = unT_loc + (out*a).T @ amT per filter ----
                ot_s = opool.tile([C, NL], f32, name="ot_s")
                ot_b = opool.tile([C, NL], f32, name="ot_b")
                nc.vector.tensor_mul(ot_s[:], po_s[:], arep_sp_sb[:])
                nc.vector.tensor_mul(ot_b[:], po_b[:], arep_bl_sb[:])
                qa = psG.tile([128, JCH, C], f32, name="qa", tag="gt")
                for j in range(JCH):
                    nc.tensor.matmul(
                        qa[:, j, :], ot_s[:, ts(j, 128)], amT_sp_sb[:],
                        start=True, stop=False,
                    )
                    nc.tensor.matmul(
                        qa[:, j, :], ot_b[:, ts(j, 128)], amT_bl_sb[:],
                        start=False, stop=True,
                    )
                ql = qpool.tile([128, JCH, C], f32, name="ql")
                nc.vector.tensor_add(ql[:], qa[:], un_sb[:])

                if t < NITER - 1:
                    bi = dram.tile([NL, C], f32, name="bi")
                    bo = dram.tile([N, C], f32, addr_space="Shared", name="bo")
                    nc.sync.dma_start(
                        bi[:].rearrange("(j p) c -> p j c", p=128), ql[:]
                    )
                    nc.gpsimd.collective_compute(
                        "AllGather",
                        mybir.AluOpType.bypass,
                        replica_groups=[list(range(R))],
                        ins=[bi[:].opt()],
                        outs=[bo[:].opt()],
                    )
                    nc.sync.dma_start(
                        q_sb[:], bo[:].rearrange("(k p) c -> p k c", p=128)
                    )
                else:
                    nc.sync.dma_start(
                        qT_out[:].rearrange("(j p) c -> p j c", p=128), ql[:]
                    )

    nc.compile()
    return nc


def _get_program():
    if "nc" not in _CACHE:
        _CACHE["nc"] = _build_program()
    return _CACHE["nc"]


def _host_prep(unaries, feat, sw, bw, compat):
    f_sp = feat[:S] / THETA_GAMMA
    f_bl = feat
    f2_sp = np.sum(f_sp * f_sp, axis=0)
    f2_bl = np.sum(f_bl * f_bl, axis=0)

    fstack_full = np.zeros((39, N), dtype=np.float32)
    fstack_full[0:S] = f_sp
    fstack_full[S] = -0.5 * f2_sp
    fstack_full[32:38] = f_bl
    fstack_full[38] = -0.5 * f2_bl

    fstack_loc_full = np.zeros((39, N), dtype=np.float32)
    fstack_loc_full[0:S] = f_sp
    fstack_loc_full[S] = 1.0
    fstack_loc_full[32:38] = f_bl
    fstack_loc_full[38] = 1.0

    a_sp = np.exp(-0.5 * f2_sp).astype(np.float32)
    a_bl = np.exp(-0.5 * f2_bl).astype(np.float32)
    arep_sp = np.broadcast_to(a_sp[None, :], (C, N)).copy()
    arep_bl = np.broadcast_to(a_bl[None, :], (C, N)).copy()

    amT_sp = np.ascontiguousarray((-(compat @ sw)).T).astype(np.float32)
    amT_bl = np.ascontiguousarray((-(compat @ bw)).T).astype(np.float32)
    qT_init = np.ascontiguousarray(unaries.T).astype(np.float32)
    return fstack_full, fstack_loc_full, arep_sp, arep_bl, amT_sp, amT_bl, qT_init


def _make_in_maps(inputs):
    unaries = np.asarray(inputs["unaries"], dtype=np.float32)
    feat = np.asarray(inputs["feat"], dtype=np.float32)
    sw = np.asarray(inputs["spatial_weights"], dtype=np.float32)
    bw = np.asarray(inputs["bilateral_weights"], dtype=np.float32)
    compat = np.asarray(inputs["compatibility_matrix"], dtype=np.float32)

    fstack_full, fstack_loc_full, arep_sp, arep_bl, amT_sp, amT_bl, qT_init = (
        _host_prep(unaries, feat, sw, bw, compat)
    )
    in_maps = []
    for r in range(R):
        jsl = slice(r * NL, (r + 1) * NL)
        in_maps.append({
            "fstack_full": fstack_full,
            "fstack_loc": np.ascontiguousarray(fstack_loc_full[:, jsl]),
            "qT_init": qT_init,
            "unT_loc": np.ascontiguousarray(qT_init[jsl]),
            "amT_sp": amT_sp,
            "amT_bl": amT_bl,
            "arep_sp": np.ascontiguousarray(arep_sp[:, jsl]),
            "arep_bl": np.ascontiguousarray(arep_bl[:, jsl]),
        })
    return in_maps


def kernel(unaries, feat, spatial_weights, bilateral_weights, compatibility_matrix):
    from concourse.bass_utils import run_bass_kernel_spmd

    in_maps = _make_in_maps({
        "unaries": unaries,
        "feat": feat,
        "spatial_weights": spatial_weights,
        "bilateral_weights": bilateral_weights,
        "compatibility_matrix": compatibility_matrix,
    })
    nc = _get_program()
    res = run_bass_kernel_spmd(nc, in_maps, core_ids=list(range(R)))

    q = np.empty((C, N), dtype=np.float32)
    for r in range(R):
        q[:, r * NL:(r + 1) * NL] = res.results[r]["qT_out"].T
    return q


# revision 5
# speedup vs baseline: 2.1894x; 1.1701x over previous
"""CRF mean-field (nn_CRF) Trainium2 kernel, SPMD over 8 NeuronCores.

Math: 5 iterations of
    p   = softmax(q, axis=classes)
    out_f = p @ K_f           for two Gaussian kernels K_f (spatial, bilateral)
    q   = unaries - compat @ (sw @ out_sp + bw @ out_bl)

Sharding: points N=8192 split column-wise across 8 cores (1024 each). Each core
rebuilds its [8192, 1024] kernel slab on the fly every iteration (cheaper than
streaming a precomputed slab from HBM): the PE computes the partial Gaussian
exponent  G'' = f_i . f_j - 0.5|f_i|^2  via an augmented feature matmul
(lhsT rows = [f ; -0.5|f|^2], rhs rows = [f ; 1]), ScalarE exponentiates
straight out of PSUM, and the tile is immediately consumed as the rhs of the
accumulating p @ K matmul. The remaining exp(-0.5|f_j|^2) factor is an exact
fp32 per-column post-scale, so rounding error only enters through terms that
average out over the 8192-point contraction.

The slab matmuls run in bf16: on TRN2 the PE clock-gate (HAM) only registers
bf16-path activity, so fp32/fp32r streams run at the cold 1.2 GHz clock
forever while a dense bf16 stream gets 2.4 GHz. bf16 also keeps 1 cyc/row.

Per iteration the cores exchange their local class distribution p (20KB bf16)
via AllGather; the iteration-1 softmax is computed on the host.
"""

import numpy as np
import ml_dtypes

C = 10          # classes
N = 8192        # points
S = 3           # spatial dims
R = 8           # cores
NL = N // R     # local points per core
KCH = N // 128  # 64 i-chunks
JCH = NL // 128  # 8 local j-chunks
NITER = 5
THETA_GAMMA = 8.0

_CACHE = {}


def _build_program():
    import concourse.mybir as mybir
    import concourse.tile as tile
    from concourse import bacc
    from concourse.bass import ts, ds

    f32 = mybir.dt.float32
    bf16 = mybir.dt.bfloat16
    nc = bacc.Bacc("TRN2", target_bir_lowering=False, debug=False, num_devices=R)

    # ---- I/O ----
    fstack_full = nc.dram_tensor("fstack_full", [39, N], f32, kind="ExternalInput")
    fstack_loc = nc.dram_tensor("fstack_loc", [39, NL], f32, kind="ExternalInput")
    p_init = nc.dram_tensor("p_init", [N, C], mybir.dt.bfloat16, kind="ExternalInput")
    unT_loc = nc.dram_tensor("unT_loc", [NL, C], f32, kind="ExternalInput")
    amT_sp = nc.dram_tensor("amT_sp", [C, C], f32, kind="ExternalInput")
    amT_bl = nc.dram_tensor("amT_bl", [C, C], f32, kind="ExternalInput")
    arep_sp = nc.dram_tensor("arep_sp", [C, NL], f32, kind="ExternalInput")
    arep_bl = nc.dram_tensor("arep_bl", [C, NL], f32, kind="ExternalInput")
    qT_out = nc.dram_tensor("qT_out", [NL, C], f32, kind="ExternalOutput")

    EXP = mybir.ActivationFunctionType.Exp

    with tile.TileContext(nc) as tc:
        with (
            tc.tile_pool(name="const", bufs=1) as const,
            tc.tile_pool(name="state", bufs=1) as state,
            tc.tile_pool(name="epool", bufs=8) as epool,
            tc.tile_pool(name="opool", bufs=2) as opool,
            tc.tile_pool(name="qpool", bufs=2) as qpool,
            tc.tile_pool(name="psG", bufs=2, space="PSUM") as psG,
            tc.tile_pool(name="psOs", bufs=1, space="PSUM") as psOs,
            tc.tile_pool(name="psOb", bufs=1, space="PSUM") as psOb,
            tc.tile_pool(name="dram", bufs=2, space="DRAM") as dram,
        ):
            # ---- load constants; cast feature stacks to bf16 on-chip ----
            ff_st = const.tile([39, N], f32, name="ff_st")
            fl_st = const.tile([39, NL], f32, name="fl_st")
            ff_sb = const.tile([39, N], bf16, name="ff_sb")
            fl_sb = const.tile([39, NL], bf16, name="fl_sb")
            amT_sp_sb = const.tile([C, C], f32, name="amT_sp_sb")
            amT_bl_sb = const.tile([C, C], f32, name="amT_bl_sb")
            arep_sp_sb = const.tile([C, NL], f32, name="arep_sp_sb")
            arep_bl_sb = const.tile([C, NL], f32, name="arep_bl_sb")
            un_sb = const.tile([128, JCH, C], f32, name="un_sb")
            nc.sync.dma_start(ff_st[:], fstack_full[:])
            nc.sync.dma_start(fl_st[:], fstack_loc[:])
            nc.vector.tensor_copy(ff_sb[:], ff_st[:])
            nc.vector.tensor_copy(fl_sb[:], fl_st[:])
            nc.sync.dma_start(amT_sp_sb[:], amT_sp[:])
            nc.sync.dma_start(amT_bl_sb[:], amT_bl[:])
            nc.sync.dma_start(arep_sp_sb[:], arep_sp[:])
            nc.sync.dma_start(arep_bl_sb[:], arep_bl[:])
            nc.sync.dma_start(
                un_sb[:], unT_loc[:].rearrange("(j p) c -> p j c", p=128)
            )

            # full class distribution (bf16), rebuilt from the gather each iter
            p_sb = state.tile([128, KCH, C], bf16, name="p_sb")
            nc.sync.dma_start(p_sb[:], p_init[:].rearrange("(k p) c -> p k c", p=128))

            # local softmax scratch
            mx_sb = state.tile([128, JCH], f32, name="mx_sb")
            sm_sb = state.tile([128, JCH], f32, name="sm_sb")
            rs_sb = state.tile([128, JCH], f32, name="rs_sb")
            el_sb = state.tile([128, JCH, C], f32, name="el_sb")

            for t in range(NITER):
                # ---- out[10, NL] = p.T-slab matmuls with fused kernel build ----
                po_s = psOs.tile([C, NL], f32, name="po_s")
                po_b = psOb.tile([C, NL], f32, name="po_b")
                for k in range(KCH):
                    for h in range(2):
                        jsl = ds(h * 512, 512)
                        gt = psG.tile([128, 1024], f32, name="gt", tag="gt")
                        nc.tensor.matmul(
                            gt[:, 0:512],
                            ff_sb[0:4, ts(k, 128)],
                            fl_sb[0:4, jsl],
                            start=True, stop=True,
                        )
                        nc.tensor.matmul(
                            gt[:, 512:1024],
                            ff_sb[32:39, ts(k, 128)],
                            fl_sb[32:39, jsl],
                            start=True, stop=True,
                        )
                        et = epool.tile([128, 1024], bf16, name="et")
                        nc.scalar.activation(et[:], gt[:], EXP, bias=0.0, scale=1.0)
                        nc.tensor.matmul(
                            po_s[:, jsl],
                            p_sb[:, k, :],
                            et[:, 0:512],
                            start=(k == 0), stop=(k == KCH - 1),
                        )
                        nc.tensor.matmul(
                            po_b[:, jsl],
                            p_sb[:, k, :],
                            et[:, 512:1024],
                            start=(k == 0), stop=(k == KCH - 1),
                        )

                # ---- q_loc.T = unT_loc + (out*a).T @ amT (stacked filters) ----
                ot_s = opool.tile([C, NL], f32, name="ot_s")
                ot_b = opool.tile([C, NL], f32, name="ot_b")
                nc.vector.tensor_mul(ot_s[:], po_s[:], arep_sp_sb[:])
                nc.vector.tensor_mul(ot_b[:], po_b[:], arep_bl_sb[:])
                qa = psG.tile([128, JCH, C], f32, name="qa", tag="gt")
                for j in range(JCH):
                    nc.tensor.matmul(
                        qa[:, j, :], ot_s[:, ts(j, 128)], amT_sp_sb[:],
                        start=True, stop=False,
                    )
                    nc.tensor.matmul(
                        qa[:, j, :], ot_b[:, ts(j, 128)], amT_bl_sb[:],
                        start=False, stop=True,
                    )
                ql = qpool.tile([128, JCH, C], f32, name="ql")
                nc.vector.tensor_add(ql[:], qa[:], un_sb[:])

                if t < NITER - 1:
                    # ---- local softmax -> p shard (bf16) -> AllGather ----
                    nc.vector.reduce_max(mx_sb[:], ql[:], axis=mybir.AxisListType.X)
                    mx_b = mx_sb[:].unsqueeze(2).broadcast_to((128, JCH, C))
                    nc.vector.tensor_sub(el_sb[:], ql[:], mx_b)
                    nc.scalar.activation(el_sb[:], el_sb[:], EXP, bias=0.0, scale=1.0)
                    nc.vector.reduce_sum(sm_sb[:], el_sb[:], axis=mybir.AxisListType.X)
                    nc.vector.reciprocal(rs_sb[:], sm_sb[:])
                    rs_b = rs_sb[:].unsqueeze(2).broadcast_to((128, JCH, C))
                    pl = qpool.tile([128, JCH, C], bf16, name="pl")
                    nc.vector.tensor_mul(pl[:], el_sb[:], rs_b)

                    bi = dram.tile([NL, C], bf16, name="bi")
                    bo = dram.tile([N, C], bf16, addr_space="Shared", name="bo")
                    nc.sync.dma_start(
                        bi[:].rearrange("(j p) c -> p j c", p=128), pl[:]
                    )
                    nc.gpsimd.collective_compute(
                        "AllGather",
                        mybir.AluOpType.bypass,
                        replica_groups=[list(range(R))],
                        ins=[bi[:].opt()],
                        outs=[bo[:].opt()],
                    )
                    nc.sync.dma_start(
                        p_sb[:], bo[:].rearrange("(k p) c -> p k c", p=128)
                    )
                else:
                    nc.sync.dma_start(
                        qT_out[:].rearrange("(j p) c -> p j c", p=128), ql[:]
                    )

    nc.compile()
    return nc


def _get_program():
    if "nc" not in _CACHE:
        _CACHE["nc"] = _build_program()
    return _CACHE["nc"]


def _host_prep(unaries, feat, sw, bw, compat):
    f_sp = feat[:S] / THETA_GAMMA
    f_bl = feat
    f2_sp = np.sum(f_sp * f_sp, axis=0)
    f2_bl = np.sum(f_bl * f_bl, axis=0)

    fstack_full = np.zeros((39, N), dtype=np.float32)
    fstack_full[0:S] = f_sp
    fstack_full[S] = -0.5 * f2_sp
    fstack_full[32:38] = f_bl
    fstack_full[38] = -0.5 * f2_bl

    fstack_loc_full = np.zeros((39, N), dtype=np.float32)
    fstack_loc_full[0:S] = f_sp
    fstack_loc_full[S] = 1.0
    fstack_loc_full[32:38] = f_bl
    fstack_loc_full[38] = 1.0

    a_sp = np.exp(-0.5 * f2_sp).astype(np.float32)
    a_bl = np.exp(-0.5 * f2_bl).astype(np.float32)
    arep_sp = np.broadcast_to(a_sp[None, :], (C, N)).copy()
    arep_bl = np.broadcast_to(a_bl[None, :], (C, N)).copy()

    amT_sp = np.ascontiguousarray((-(compat @ sw)).T).astype(np.float32)
    amT_bl = np.ascontiguousarray((-(compat @ bw)).T).astype(np.float32)

    qT_init = np.ascontiguousarray(unaries.T).astype(np.float32)
    # iteration-1 softmax on the host
    mx = unaries.max(axis=0, keepdims=True)
    e = np.exp(unaries - mx, dtype=np.float32)
    p0 = (e / e.sum(axis=0, keepdims=True)).astype(np.float32)
    p0T = np.ascontiguousarray(p0.T).astype(ml_dtypes.bfloat16)
    return fstack_full, fstack_loc_full, arep_sp, arep_bl, amT_sp, amT_bl, qT_init, p0T


def _make_in_maps(inputs):
    unaries = np.asarray(inputs["unaries"], dtype=np.float32)
    feat = np.asarray(inputs["feat"], dtype=np.float32)
    sw = np.asarray(inputs["spatial_weights"], dtype=np.float32)
    bw = np.asarray(inputs["bilateral_weights"], dtype=np.float32)
    compat = np.asarray(inputs["compatibility_matrix"], dtype=np.float32)

    fstack_full, fstack_loc_full, arep_sp, arep_bl, amT_sp, amT_bl, qT_init, p0T = (
        _host_prep(unaries, feat, sw, bw, compat)
    )
    in_maps = []
    for r in range(R):
        jsl = slice(r * NL, (r + 1) * NL)
        in_maps.append({
            "fstack_full": fstack_full,
            "fstack_loc": np.ascontiguousarray(fstack_loc_full[:, jsl]),
            "p_init": p0T,
            "unT_loc": np.ascontiguousarray(qT_init[jsl]),
            "amT_sp": amT_sp,
            "amT_bl": amT_bl,
            "arep_sp": np.ascontiguousarray(arep_sp[:, jsl]),
            "arep_bl": np.ascontiguousarray(arep_bl[:, jsl]),
        })
    return in_maps


def kernel(unaries, feat, spatial_weights, bilateral_weights, compatibility_matrix):
    from concourse.bass_utils import run_bass_kernel_spmd

    in_maps = _make_in_maps({
        "unaries": unaries,
        "feat": feat,
        "spatial_weights": spatial_weights,
        "bilateral_weights": bilateral_weights,
        "compatibility_matrix": compatibility_matrix,
    })
    nc = _get_program()
    res = run_bass_kernel_spmd(nc, in_maps, core_ids=list(range(R)))

    q = np.empty((C, N), dtype=np.float32)
    for r in range(R):
        q[:, r * NL:(r + 1) * NL] = res.results[r]["qT_out"].T
    return q


# revision 6
# speedup vs baseline: 3.2471x; 1.4831x over previous
"""CRF mean-field (nn_CRF) Trainium2 kernel, SPMD over 8 NeuronCores.

Math: 5 iterations of
    p   = softmax(q, axis=classes)
    out_f = p @ K_f           for two Gaussian kernels K_f (spatial, bilateral)
    q   = unaries - compat @ (sw @ out_sp + bw @ out_bl)

Sharding: points N=8192 split column-wise across 8 cores (1024 each). Each core
builds its [8192, 1024] kernel slab tile-by-tile: the PE computes the partial
Gaussian exponent  G'' = f_i . f_j - 0.5|f_i|^2  via an augmented feature
matmul (lhsT rows = [f ; -0.5|f|^2], rhs rows = [f ; 1]), ScalarE
exponentiates straight out of PSUM into a bf16 tile, and the tile is
immediately consumed as the rhs of the accumulating p @ K matmul. The
remaining exp(-0.5|f_j|^2) factor is an exact fp32 per-column post-scale, so
rounding error only enters through terms that average out over the
8192-point contraction.

The kernel slab is constant across iterations: iteration 1 additionally spills
each bf16 kernel tile to an HBM cache, and iterations 2-5 stream 3/4 of the
tiles back via DMA while rebuilding the rest (PE+ACT), balancing HBM bandwidth
against ScalarE exp throughput.

Slab matmuls run in bf16: on TRN2 the PE clock-gate (HAM) only registers
bf16-path activity, so fp32/fp32r streams run at the cold 1.2 GHz clock; bf16
gets 2.4 GHz. The two filters' p @ K matmuls go to different PSUM column
groups (tile_position=(0,32)) so they run concurrently on the array.

Per iteration the cores exchange their local class distribution p (20KB bf16)
via AllGather; the iteration-1 softmax is computed on the host.
"""

import numpy as np
import ml_dtypes

C = 10          # classes
N = 8192        # points
S = 3           # spatial dims
R = 8           # cores
NL = N // R     # local points per core
KCH = N // 128  # 64 i-chunks
JCH = NL // 128  # 8 local j-chunks
NITER = 5
THETA_GAMMA = 8.0
REBUILD_EVERY = 4   # in iters 2+, rebuild every 4th tile; stream the rest

_CACHE = {}


def _build_program():
    import concourse.mybir as mybir
    import concourse.tile as tile
    from concourse import bacc
    from concourse.bass import ts, ds

    f32 = mybir.dt.float32
    bf16 = mybir.dt.bfloat16
    nc = bacc.Bacc("TRN2", target_bir_lowering=False, debug=False, num_devices=R)

    # ---- I/O ----
    fstack_full = nc.dram_tensor("fstack_full", [39, N], f32, kind="ExternalInput")
    fstack_loc = nc.dram_tensor("fstack_loc", [39, NL], f32, kind="ExternalInput")
    p_init = nc.dram_tensor("p_init", [N, C], bf16, kind="ExternalInput")
    unT_loc = nc.dram_tensor("unT_loc", [NL, C], f32, kind="ExternalInput")
    amT_sp = nc.dram_tensor("amT_sp", [C, C], f32, kind="ExternalInput")
    amT_bl = nc.dram_tensor("amT_bl", [C, C], f32, kind="ExternalInput")
    arep_sp = nc.dram_tensor("arep_sp", [C, NL], f32, kind="ExternalInput")
    arep_bl = nc.dram_tensor("arep_bl", [C, NL], f32, kind="ExternalInput")
    qT_out = nc.dram_tensor("qT_out", [NL, C], f32, kind="ExternalOutput")

    EXP = mybir.ActivationFunctionType.Exp

    with tile.TileContext(nc) as tc:
        with (
            tc.tile_pool(name="const", bufs=1) as const,
            tc.tile_pool(name="state", bufs=1) as state,
            tc.tile_pool(name="epool", bufs=8) as epool,
            tc.tile_pool(name="opool", bufs=2) as opool,
            tc.tile_pool(name="qpool", bufs=2) as qpool,
            tc.tile_pool(name="psG", bufs=3, space="PSUM") as psG,
            tc.tile_pool(name="psO", bufs=1, space="PSUM") as psO,
            tc.tile_pool(name="dram", bufs=2, space="DRAM") as dram,
            tc.tile_pool(name="cache", bufs=1, space="DRAM") as cache,
        ):
            # ---- load constants; cast feature stacks to bf16 on-chip ----
            ff_st = const.tile([39, N], f32, name="ff_st")
            fl_st = const.tile([39, NL], f32, name="fl_st")
            ff_sb = const.tile([39, N], bf16, name="ff_sb")
            fl_sb = const.tile([39, NL], bf16, name="fl_sb")
            amT_sp_sb = const.tile([C, C], f32, name="amT_sp_sb")
            amT_bl_sb = const.tile([C, C], f32, name="amT_bl_sb")
            arep_sp_sb = const.tile([C, NL], f32, name="arep_sp_sb")
            arep_bl_sb = const.tile([C, NL], f32, name="arep_bl_sb")
            un_sb = const.tile([128, JCH, C], f32, name="un_sb")
            nc.sync.dma_start(ff_st[:], fstack_full[:])
            nc.sync.dma_start(fl_st[:], fstack_loc[:])
            nc.vector.tensor_copy(ff_sb[:], ff_st[:])
            nc.vector.tensor_copy(fl_sb[:], fl_st[:])
            nc.sync.dma_start(amT_sp_sb[:], amT_sp[:])
            nc.sync.dma_start(amT_bl_sb[:], amT_bl[:])
            nc.sync.dma_start(arep_sp_sb[:], arep_sp[:])
            nc.sync.dma_start(arep_bl_sb[:], arep_bl[:])
            nc.sync.dma_start(
                un_sb[:], unT_loc[:].rearrange("(j p) c -> p j c", p=128)
            )

            # HBM cache of the bf16 kernel tiles (constant across iterations)
            ecache = cache.tile([KCH, 2, 128, 1024], bf16, name="ecache")

            # full class distribution (bf16), rebuilt from the gather each iter
            p_sb = state.tile([128, KCH, C], bf16, name="p_sb")
            nc.sync.dma_start(p_sb[:], p_init[:].rearrange("(k p) c -> p k c", p=128))

            # local softmax scratch
            mx_sb = state.tile([128, JCH], f32, name="mx_sb")
            sm_sb = state.tile([128, JCH], f32, name="sm_sb")
            rs_sb = state.tile([128, JCH], f32, name="rs_sb")
            el_sb = state.tile([128, JCH, C], f32, name="el_sb")

            for t in range(NITER):
                # ---- out[10, NL] = p.T-slab matmuls with fused kernel build ----
                # po packed by column group: spatial @ partitions 0-9,
                # bilateral @ partitions 32-41 (concurrent on the PE array).
                po = psO.tile([32 + C, NL], f32, name="po")
                for k in range(KCH):
                    for h in range(2):
                        u = k * 2 + h
                        jsl = ds(h * 512, 512)
                        rebuild = (t == 0) or (u % REBUILD_EVERY == 0)
                        et = epool.tile([128, 1024], bf16, name="et")
                        if rebuild:
                            gt = psG.tile([128, 1024], f32, name="gt", tag="gt")
                            nc.tensor.matmul(
                                gt[:, 0:512],
                                ff_sb[0:4, ts(k, 128)],
                                fl_sb[0:4, jsl],
                                start=True, stop=True,
                            )
                            nc.tensor.matmul(
                                gt[:, 512:1024],
                                ff_sb[32:39, ts(k, 128)],
                                fl_sb[32:39, jsl],
                                start=True, stop=True,
                            )
                            nc.scalar.activation(
                                et[:], gt[:], EXP, bias=0.0, scale=1.0
                            )
                            if t == 0 and u % REBUILD_EVERY != 0:
                                nc.sync.dma_start(ecache[k, h], et[:])
                        else:
                            nc.sync.dma_start(et[:], ecache[k, h])
                        nc.tensor.matmul(
                            po[0:C, jsl],
                            p_sb[:, k, :],
                            et[:, 0:512],
                            start=(k == 0), stop=(k == KCH - 1),
                        )
                        nc.tensor.matmul(
                            po[32:32 + C, jsl],
                            p_sb[:, k, :],
                            et[:, 512:1024],
                            tile_position=(0, 32),
                            start=(k == 0), stop=(k == KCH - 1),
                        )

                # ---- q_loc.T = unT_loc + (out*a).T @ amT per filter ----
                ot_s = opool.tile([C, NL], f32, name="ot_s")
                ot_b = opool.tile([C, NL], f32, name="ot_b")
                nc.vector.tensor_mul(ot_s[:], po[0:C, :], arep_sp_sb[:])
                nc.vector.tensor_mul(ot_b[:], po[32:32 + C, :], arep_bl_sb[:])
                qa = psG.tile([128, JCH, C], f32, name="qa", tag="gt")
                for j in range(JCH):
                    nc.tensor.matmul(
                        qa[:, j, :], ot_s[:, ts(j, 128)], amT_sp_sb[:],
                        start=True, stop=False,
                    )
                    nc.tensor.matmul(
                        qa[:, j, :], ot_b[:, ts(j, 128)], amT_bl_sb[:],
                        start=False, stop=True,
                    )
                ql = qpool.tile([128, JCH, C], f32, name="ql")
                nc.vector.tensor_add(ql[:], qa[:], un_sb[:])

                if t < NITER - 1:
                    # ---- local softmax -> p shard (bf16) -> AllGather ----
                    nc.vector.reduce_max(mx_sb[:], ql[:], axis=mybir.AxisListType.X)
                    mx_b = mx_sb[:].unsqueeze(2).broadcast_to((128, JCH, C))
                    nc.vector.tensor_sub(el_sb[:], ql[:], mx_b)
                    nc.scalar.activation(el_sb[:], el_sb[:], EXP, bias=0.0, scale=1.0)
                    nc.vector.reduce_sum(sm_sb[:], el_sb[:], axis=mybir.AxisListType.X)
                    nc.vector.reciprocal(rs_sb[:], sm_sb[:])
                    rs_b = rs_sb[:].unsqueeze(2).broadcast_to((128, JCH, C))
                    pl = qpool.tile([128, JCH, C], bf16, name="pl")
                    nc.vector.tensor_mul(pl[:], el_sb[:], rs_b)

                    bi = dram.tile([NL, C], bf16, name="bi")
                    bo = dram.tile([N, C], bf16, addr_space="Shared", name="bo")
                    nc.sync.dma_start(
                        bi[:].rearrange("(j p) c -> p j c", p=128), pl[:]
                    )
                    nc.gpsimd.collective_compute(
                        "AllGather",
                        mybir.AluOpType.bypass,
                        replica_groups=[list(range(R))],
                        ins=[bi[:].opt()],
                        outs=[bo[:].opt()],
                    )
                    nc.sync.dma_start(
                        p_sb[:], bo[:].rearrange("(k p) c -> p k c", p=128)
                    )
                else:
                    nc.sync.dma_start(
                        qT_out[:].rearrange("(j p) c -> p j c", p=128), ql[:]
                    )

    nc.compile()
    return nc


def _get_program():
    if "nc" not in _CACHE:
        _CACHE["nc"] = _build_program()
    return _CACHE["nc"]


def _host_prep(unaries, feat, sw, bw, compat):
    f_sp = feat[:S] / THETA_GAMMA
    f_bl = feat
    f2_sp = np.sum(f_sp * f_sp, axis=0)
    f2_bl = np.sum(f_bl * f_bl, axis=0)

    fstack_full = np.zeros((39, N), dtype=np.float32)
    fstack_full[0:S] = f_sp
    fstack_full[S] = -0.5 * f2_sp
    fstack_full[32:38] = f_bl
    fstack_full[38] = -0.5 * f2_bl

    fstack_loc_full = np.zeros((39, N), dtype=np.float32)
    fstack_loc_full[0:S] = f_sp
    fstack_loc_full[S] = 1.0
    fstack_loc_full[32:38] = f_bl
    fstack_loc_full[38] = 1.0

    a_sp = np.exp(-0.5 * f2_sp).astype(np.float32)
    a_bl = np.exp(-0.5 * f2_bl).astype(np.float32)
    arep_sp = np.broadcast_to(a_sp[None, :], (C, N)).copy()
    arep_bl = np.broadcast_to(a_bl[None, :], (C, N)).copy()

    amT_sp = np.ascontiguousarray((-(compat @ sw)).T).astype(np.float32)
    amT_bl = np.ascontiguousarray((-(compat @ bw)).T).astype(np.float32)

    qT_init = np.ascontiguousarray(unaries.T).astype(np.float32)
    # iteration-1 softmax on the host
    mx = unaries.max(axis=0, keepdims=True)
    e = np.exp(unaries - mx, dtype=np.float32)
    p0 = (e / e.sum(axis=0, keepdims=True)).astype(np.float32)
    p0T = np.ascontiguousarray(p0.T).astype(ml_dtypes.bfloat16)
    return fstack_full, fstack_loc_full, arep_sp, arep_bl, amT_sp, amT_bl, qT_init, p0T


def _make_in_maps(inputs):
    unaries = np.asarray(inputs["unaries"], dtype=np.float32)
    feat = np.asarray(inputs["feat"], dtype=np.float32)
    sw = np.asarray(inputs["spatial_weights"], dtype=np.float32)
    bw = np.asarray(inputs["bilateral_weights"], dtype=np.float32)
    compat = np.asarray(inputs["compatibility_matrix"], dtype=np.float32)

    fstack_full, fstack_loc_full, arep_sp, arep_bl, amT_sp, amT_bl, qT_init, p0T = (
        _host_prep(unaries, feat, sw, bw, compat)
    )
    in_maps = []
    for r in range(R):
        jsl = slice(r * NL, (r + 1) * NL)
        in_maps.append({
            "fstack_full": fstack_full,
            "fstack_loc": np.ascontiguousarray(fstack_loc_full[:, jsl]),
            "p_init": p0T,
            "unT_loc": np.ascontiguousarray(qT_init[jsl]),
            "amT_sp": amT_sp,
            "amT_bl": amT_bl,
            "arep_sp": np.ascontiguousarray(arep_sp[:, jsl]),
            "arep_bl": np.ascontiguousarray(arep_bl[:, jsl]),
        })
    return in_maps


def kernel(unaries, feat, spatial_weights, bilateral_weights, compatibility_matrix):
    from concourse.bass_utils import run_bass_kernel_spmd

    in_maps = _make_in_maps({
        "unaries": unaries,
        "feat": feat,
        "spatial_weights": spatial_weights,
        "bilateral_weights": bilateral_weights,
        "compatibility_matrix": compatibility_matrix,
    })
    nc = _get_program()
    res = run_bass_kernel_spmd(nc, in_maps, core_ids=list(range(R)))

    q = np.empty((C, N), dtype=np.float32)
    for r in range(R):
        q[:, r * NL:(r + 1) * NL] = res.results[r]["qT_out"].T
    return q
